# revision 43
# baseline (speedup 1.0000x reference)
"""Trainium2 Bass kernel for an involution Bottleneck block (B=2, Cin=256,
Cmid=64, Cout=256, H=W=56, K=15, G=4).

Sharding: 8 cores = 2 batches x 4 H-quarters (14 output rows each). Each core
receives a zero-padded input halo [256, 28, 70] (7 rows/cols each side), so no
inter-core communication is needed (halo compute is redundant).

Per-core pipeline (channels on SBUF partitions, pixels on free dim):
  conv1 1x1 (PE, bf16) -> BN+ReLU (ACT) -> out1 [64, 28x70] bf16, plus a copy
    shifted by +7 rows at partitions 64:128 and a +1-col copy (odd-kx align).
  reduce 1x1 (PE) -> BN+ReLU (ACT) -> r [16, 784] bf16 (+ ones row for bias).
  involution as 120 tap-PAIRS (ky, ky+7): one span matmul with M=128 columns
    (64 per tap, Ws columns 16x-expanded, bias via the ones-row at K=17)
    yields the per-pixel kernels for both taps stacked on partitions; ACT
    evicts 4 PSUM banks (2 pairs x 2 pixel-halves) per op; DVE multiplies by
    the shifted out1 window with a 4D AP covering both pixel halves (bf16 2x)
    and accumulates. Partition halves hold ky-groups 0-6 / 7-13; one
    DMA + add merges them at the end, landing directly in conv3's rhs layout.
  BN+ReLU (ACT) -> conv3 1x1 (PE) -> BN (ACT) -> +residual (DVE) -> ReLU (ACT)
"""

import sys, types
sys.path.insert(0, "/opt/trn_rl_repo")

import numpy as np
import ml_dtypes
from contextlib import ExitStack

import concourse.bass as bass
import concourse.mybir as mybir
import concourse.tile as tile
from concourse import bacc
from concourse.bass import ts
from concourse.bass_utils import run_bass_kernel_spmd

BF16 = mybir.dt.bfloat16
F32 = mybir.dt.float32
AF = mybir.ActivationFunctionType

K = 15
G = 4
GC = 16
PAD = 7
CIN = 256
CMID = 64
RED = 16
COUT = 256
H = 56
W = 56
B = 2
HB = 14            # output rows per core
HP = HB + 2 * PAD  # 28 padded rows
WP = W + 2 * PAD   # 70 padded cols
NP = HP * WP       # 1960
HH = HB // 2       # 7 rows per half-block
NF = HH * W        # 392 pixels per half-block
NPAIR = 7 * K + 8  # 105 (ky,ky+7) pairs + 8 row-14 (kx,kx+1) pairs = 113
NQ = (NPAIR + 1) // 2  # 57 groups (2 pairs each; last group has 1 pair)
WCH = 38           # wse pairs per 32-partition chunk

_PROGRAM = None  # (nc, names) cache


def _build_program():
    nc = bacc.Bacc(None, target_bir_lowering=False, debug=False)
    with tile.TileContext(nc) as tc, ExitStack() as ctx:
        dram = ctx.enter_context(tc.tile_pool(name="dram", bufs=1, space="DRAM"))
        xb_d = dram.tile([CIN, NP], BF16, kind="ExternalInput", name="xb")
        xr_d = dram.tile([COUT, HB * W], F32, kind="ExternalInput", name="xr")
        w1t_d = dram.tile([CIN, CMID], BF16, kind="ExternalInput", name="w1t")
        wrt_d = dram.tile([CMID, RED], BF16, kind="ExternalInput", name="wrt")
        wse_d = dram.tile([81, WCH * 128], BF16, kind="ExternalInput", name="wse")
        w3t_d = dram.tile([CMID, COUT], BF16, kind="ExternalInput", name="w3t")
        vec_d = dram.tile([128, 10], F32, kind="ExternalInput", name="vecs")
        ones_d = dram.tile([1, 2 * NF], BF16, kind="ExternalInput", name="ones")
        y_d = dram.tile([COUT, HB * W], F32, kind="ExternalOutput", name="y")

        xpool = ctx.enter_context(tc.tile_pool(name="xin", bufs=1))
        xb = xpool.tile([128, 2, NP], BF16)
        nc.sync.dma_start(out=xb[:], in_=xb_d[:].rearrange("(c p) n -> p c n", p=128))

        wpool = ctx.enter_context(tc.tile_pool(name="weights", bufs=1))
        w1t = wpool.tile([128, 2, CMID], BF16)
        nc.sync.dma_start(out=w1t[:], in_=w1t_d[:].rearrange("(c p) m -> p c m", p=128))
        wrt = wpool.tile([CMID, RED], BF16)
        nc.sync.dma_start(out=wrt[:], in_=wrt_d[:])
        # span weights in 4 vertical chunks at partitions 0/32/64/96 so the
        # DMA spans 113 partitions and fans out across the HWDGE queues
        wse = wpool.tile([81, WCH * 128], BF16)
        nc.sync.dma_start(out=wse[:], in_=wse_d[:])
        w3t = wpool.tile([CMID, COUT], BF16)
        nc.sync.dma_start(out=w3t[:], in_=w3t_d[:])
        vecs = wpool.tile([128, 10], F32)
        nc.sync.dma_start(out=vecs[:], in_=vec_d[:])

        xr = xpool.tile([128, 2, HB * W], F32)
        nc.sync.dma_start(out=xr[:], in_=xr_d[:].rearrange("(c p) n -> p c n", p=128))

        opool = ctx.enter_context(tc.tile_pool(name="out1", bufs=1))
        out1p = opool.tile([128, NP], BF16)
        out1q = opool.tile([128, NP], BF16)
        out1r = opool.tile([128, NP], BF16)  # row-14 pairs: [0:64]=blockA, [64:128]=blockA<<1col

        # conv1: out1 = relu(g1 * (W1 @ x) + b1) over all 28x70 padded pixels
        with tc.tile_pool(name="p1", bufs=4, space="PSUM") as p1:
            for j in (1, 2, 3, 0):
                ps = p1.tile([CMID, 490], F32, tag="ps1")
                nc.tensor.matmul(ps[:], w1t[:, 0, :], xb[:, 0, ts(j, 490)],
                                 start=True, stop=False)
                nc.tensor.matmul(ps[:], w1t[:, 1, :], xb[:, 1, ts(j, 490)],
                                 start=False, stop=True)
                nc.scalar.activation(out1p[0:CMID, ts(j, 490)], ps[:], AF.Relu,
                                     bias=vecs[0:CMID, 1:2], scale=vecs[0:CMID, 0:1])

        # rows 7..27 duplicated at partitions 64:128 (the +7-row tap shift);
        # zero the unwritten tail there so zero-padded taps read 0, not junk.
        DUPW = (HP - HH) * WP  # 1470
        nc.vector.memset(out1p[CMID:128, DUPW:NP], 0.0)
        nc.vector.memset(out1q[CMID:128, DUPW - 1:NP], 0.0)
        nc.vector.memset(out1q[0:CMID, NP - 1:NP], 0.0)
        nc.sync.dma_start(out=out1p[CMID:128, 0:DUPW],
                          in_=out1p[0:CMID, HH * WP:NP])
        o3 = out1p[:].rearrange("p (h w) -> p h w", w=WP)
        o3q = out1q[:].rearrange("p (h w) -> p h w", w=WP)
        o3r = out1r[:].rearrange("p (h w) -> p h w", w=WP)

        spool = ctx.enter_context(tc.tile_pool(name="stage", bufs=1))
        r_sb = spool.tile([81, 2 * NF], BF16)
        # ones row (span bias via the K dimension); DMA since engines cannot
        # address a single partition at offset 16
        nc.sync.dma_start(out=r_sb[RED:RED + 1, :], in_=ones_d[:])

        # reduce: r = relu(gr * (Wr @ out1_central) + br), central 14x56 pixels
        with tc.tile_pool(name="pr", bufs=2, space="PSUM") as pr:
            for hhalf in range(2):
                ps = pr.tile([RED, NF], F32, tag="psr")
                nc.tensor.matmul(ps[:], wrt[:],
                                 o3[0:CMID, PAD + HH * hhalf:PAD + HH * (hhalf + 1), PAD:PAD + W],
                                 start=True, stop=True)
                nc.scalar.activation(r_sb[0:RED, ts(hhalf, NF)], ps[:], AF.Relu,
                                     bias=vecs[0:RED, 3:4], scale=vecs[0:RED, 2:3])

        # replicate r (+ones row) at partitions 32/64/96 to match the span
        # lhsT chunks' base partitions
        for c in range(1, 3):
            nc.sync.dma_start(out=r_sb[32 * c:32 * c + RED + 1, :],
                              in_=r_sb[0:RED + 1, :])

        # +1-col shifted copies (odd-kx 4B alignment) are same-partition, so
        # they run on ACT; placed after the reduce so r is produced first
        nc.vector.tensor_copy(out1q[0:CMID, 0:NP - 1], out1p[0:CMID, 1:NP])
        nc.vector.tensor_copy(out1q[CMID:128, 0:DUPW - 1], out1p[CMID:128, 1:DUPW])
        # row-14 pair source: partitions 64:128 hold block A shifted +1 col
        nc.vector.memset(out1r[CMID:128, NP - 1:NP], 0.0)
        nc.vector.tensor_copy(out1r[0:CMID, :], out1p[0:CMID, :])
        nc.sync.dma_start(out=out1r[CMID:128, 0:NP - 1], in_=out1p[0:CMID, 1:NP])

        # involution: 60 quad groups x (2 pairs x 2 pixel-halves)
        acc = None
        with tc.tile_pool(name="sp", bufs=2, space="PSUM") as sp, \
             tc.tile_pool(name="we", bufs=5) as we_pool, \
             tc.tile_pool(name="prod", bufs=4) as prod_pool, \
             tc.tile_pool(name="accp", bufs=4) as acc_pool:
            for q in range(NQ):
                npr = 2 if 2 * q + 1 < NPAIR else 1
                ps = sp.tile([128, 2048], F32, tag="spanps")
                for i in range(npr):
                    pi = 2 * q + i
                    wc, wo = pi // WCH, pi % WCH
                    lhsT = wse[32 * wc:32 * wc + RED + 1, ts(wo, 128)]
                    rr = r_sb[32 * wc:32 * wc + RED + 1, :]
                    nc.tensor.matmul(ps[:, i * 1024 + 0:i * 1024 + NF],
                                     lhsT, rr[:, 0:NF], start=True, stop=True)
                    nc.tensor.matmul(ps[:, i * 1024 + 512:i * 1024 + 512 + NF],
                                     lhsT, rr[:, NF:2 * NF], start=True, stop=True)
                we = we_pool.tile([128, 2 * npr, NF], BF16, tag="we")
                nc.scalar.activation(
                    we[:], ps[:].rearrange("p (q x) -> p q x", x=512)[:, 0:2 * npr, 0:NF],
                    AF.Copy, scale=1.0)
                prod = prod_pool.tile([128, 2, 2 * NF], BF16, tag="prod")
                for i in range(npr):
                    pi = 2 * q + i
                    if pi < 7 * K:
                        ky, kx = pi // K, pi % K
                        if kx % 2 == 0:
                            src_ = o3[:, ky:ky + 2 * HH, kx:kx + W]
                        else:
                            src_ = o3q[:, ky:ky + 2 * HH, kx - 1:kx - 1 + W]
                    else:
                        kx = 2 * (pi - 7 * K)  # row-14 pair (14,kx)+(14,kx+1)
                        src_ = o3r[:, 14:14 + 2 * HH, kx:kx + W]
                    nc.vector.tensor_mul(
                        prod[:, i, :].rearrange("p (b h w) -> p b h w", b=2, w=W),
                        we[:, 2 * i:2 * i + 2, :].rearrange("p b (h w) -> p b h w", w=W),
                        src_.rearrange("p (b h) w -> p b h w", b=2))
                if npr == 2:
                    s = acc_pool.tile([128, 2 * NF], BF16, tag="s")
                    nc.vector.tensor_add(s[:], prod[:, 0, :], prod[:, 1, :])
                else:
                    s = prod[:, 0, :]
                if acc is None:
                    acc = s
                else:
                    na = acc_pool.tile([128, 2 * NF], BF16, tag="acc")
                    nc.vector.tensor_add(na[:], acc[:], s[:])
                    acc = na

        # merge ky-groups: inv[c] = acc[c] + acc[64+c], pipelined by halves,
        # landing directly in conv3's rhs layout [64, 784]
        tmp = spool.tile([CMID, 2 * NF], BF16)
        inv = spool.tile([CMID, 2 * NF], BF16)
        out2f = spool.tile([CMID, 2 * NF], BF16)
        for h in range(2):
            nc.sync.dma_start(out=tmp[:, ts(h, NF)], in_=acc[CMID:128, ts(h, NF)])
            nc.vector.tensor_add(inv[:, ts(h, NF)], acc[0:CMID, ts(h, NF)],
                                 tmp[:, ts(h, NF)])
            nc.scalar.activation(out2f[:, ts(h, NF)], inv[:, ts(h, NF)], AF.Relu,
                                 bias=vecs[0:CMID, 5:6], scale=vecs[0:CMID, 4:5])

        # conv3 + BN3 + residual + relu
        with tc.tile_pool(name="p3", bufs=2, space="PSUM") as p3, \
             tc.tile_pool(name="ypool", bufs=2) as ypool:
            for nh in range(2):
                for mc in range(2):
                    ps = p3.tile([128, NF], F32, tag="ps3")
                    nc.tensor.matmul(ps[:], w3t[:, ts(mc, 128)], out2f[:, ts(nh, NF)],
                                     start=True, stop=True)
                    t3 = ypool.tile([128, NF], F32, tag="t3")
                    nc.scalar.activation(t3[:], ps[:], AF.Identity,
                                         bias=vecs[:, 8 + mc:9 + mc],
                                         scale=vecs[:, 6 + mc:7 + mc])
                    ys = ypool.tile([128, NF], F32, tag="ys")
                    nc.vector.tensor_add(ys[:], t3[:], xr[:, mc, ts(nh, NF)])
                    yr = ypool.tile([128, NF], F32, tag="yr")
                    nc.scalar.activation(yr[:], ys[:], AF.Relu, scale=1.0)
                    nc.sync.dma_start(
                        out=y_d[:].rearrange("(c p) n -> p c n", p=128)[:, mc, ts(nh, NF)],
                        in_=yr[:])

    nc.compile()
    names = dict(xb=xb_d.name, xr=xr_d.name, w1t=w1t_d.name, wrt=wrt_d.name,
                 wse=wse_d.name, w3t=w3t_d.name, vecs=vec_d.name,
                 ones=ones_d.name, y=y_d.name)
    return nc, names


def _get_program():
    global _PROGRAM
    if _PROGRAM is None:
        _PROGRAM = _build_program()
    return _PROGRAM


def _bf16(a):
    return np.asarray(a, dtype=np.float32).astype(ml_dtypes.bfloat16)


def kernel(x, W1, g1, b1, Wr, gr, br, Ws, bs, g2, b2, W3, g3, b3,
           _want_results=False, _trace=False):
    x = np.asarray(x, dtype=np.float32)
    nc, names = _get_program()

    w1t = _bf16(np.asarray(W1).T)                      # [256, 64]
    wrt = _bf16(np.asarray(Wr).T)                      # [64, 16]
    w3t = _bf16(np.asarray(W3).T)                      # [64, 256]

    # span weights, 16x channel-expanded, tap-paired (ky, ky+7), bias row 16.
    # wse[:, pi*128 + j]: j<64 -> tap (ky,kx), j>=64 -> tap (ky+7,kx) (zeros
    # for the ky=14 solo row).  pi = ky*15 + kx, ky in 0..7.
    Ws = np.asarray(Ws, dtype=np.float32)              # [900, 16]
    bs = np.asarray(bs, dtype=np.float32)              # [900]
    gidx = np.arange(CMID) // GC                       # [64]
    WsT = Ws.reshape(G, K * K, RED)                    # [g, k, rho]
    bsr = bs.reshape(G, K * K)
    wse = np.zeros((RED + 1, NPAIR, 128), dtype=np.float32)
    for pi in range(NPAIR):
        if pi < 7 * K:
            ky, kx = pi // K, pi % K
            k1, k2 = ky * K + kx, (ky + 7) * K + kx
        else:
            kx = 2 * (pi - 7 * K)
            k1 = 14 * K + kx
            k2 = 14 * K + kx + 1 if kx + 1 < K else None
        wse[0:RED, pi, 0:CMID] = WsT[gidx, k1, :].T
        wse[RED, pi, 0:CMID] = bsr[gidx, k1]
        if k2 is not None:
            wse[0:RED, pi, CMID:128] = WsT[gidx, k2, :].T
            wse[RED, pi, CMID:128] = bsr[gidx, k2]
    wse4 = np.zeros((81, WCH * 128), dtype=np.float32)
    for pi in range(NPAIR):
        wc, wo = pi // WCH, pi % WCH
        wse4[32 * wc:32 * wc + RED + 1, wo * 128:(wo + 1) * 128] = wse[:, pi, :]
    wse = _bf16(wse4)

    vecs = np.zeros((128, 10), dtype=np.float32)
    vecs[0:CMID, 0] = g1
    vecs[0:CMID, 1] = b1
    vecs[0:RED, 2] = gr
    vecs[0:RED, 3] = br
    vecs[0:CMID, 4] = g2
    vecs[0:CMID, 5] = b2
    vecs[:, 6] = np.asarray(g3)[0:128]
    vecs[:, 7] = np.asarray(g3)[128:256]
    vecs[:, 8] = np.asarray(b3)[0:128]
    vecs[:, 9] = np.asarray(b3)[128:256]

    in_maps = []
    core_geom = []
    for core in range(8):
        b = core // 4
        h0 = (core % 4) * HB
        xpad = np.zeros((CIN, HP, WP), dtype=np.float32)
        lo, hi = h0 - PAD, h0 + HB + PAD
        slo, shi = max(lo, 0), min(hi, H)
        xpad[:, slo - lo:shi - lo, PAD:PAD + W] = x[b, :, slo:shi, :]
        xbc = _bf16(xpad).reshape(CIN, NP)
        xrc = np.ascontiguousarray(x[b, :, h0:h0 + HB, :]).reshape(COUT, HB * W)
        in_maps.append({
            names["xb"]: xbc,
            names["xr"]: xrc,
            names["w1t"]: w1t,
            names["wrt"]: wrt,
            names["wse"]: wse,
            names["w3t"]: w3t,
            names["vecs"]: vecs,
            names["ones"]: np.ones((1, 2 * NF), dtype=np.float32).astype(ml_dtypes.bfloat16),
        })
        core_geom.append((b, h0))

    res = run_bass_kernel_spmd(nc, in_maps, list(range(8)), trace=_trace)

    y = np.empty((B, COUT, H, W), dtype=np.float32)
    for core, (b, h0) in enumerate(core_geom):
        y[b, :, h0:h0 + HB, :] = res.results[core][names["y"]].reshape(COUT, HB, W)
    if _want_results:
        return y, res
    return y


# revision 44
# speedup vs baseline: 1.0076x; 1.0076x over previous
"""Trainium2 Bass kernel for an involution Bottleneck block (B=2, Cin=256,
Cmid=64, Cout=256, H=W=56, K=15, G=4).

Sharding: 8 cores = 2 batches x 4 H-quarters (14 output rows each). Each core
receives a zero-padded input halo [256, 28, 70] (7 rows/cols each side), so no
inter-core communication is needed (halo compute is redundant).

Per-core pipeline (channels on SBUF partitions, pixels on free dim):
  conv1 1x1 (PE, bf16) -> BN+ReLU (ACT) -> out1 [64, 28x70] bf16, plus a copy
    shifted by +7 rows at partitions 64:128 and a +1-col copy (odd-kx align).
  reduce 1x1 (PE) -> BN+ReLU (ACT) -> r [16, 784] bf16 (+ ones row for bias).
  involution as 120 tap-PAIRS (ky, ky+7): one span matmul with M=128 columns
    (64 per tap, Ws columns 16x-expanded, bias via the ones-row at K=17)
    yields the per-pixel kernels for both taps stacked on partitions; ACT
    evicts 4 PSUM banks (2 pairs x 2 pixel-halves) per op; DVE multiplies by
    the shifted out1 window with a 4D AP covering both pixel halves (bf16 2x)
    and accumulates. Partition halves hold ky-groups 0-6 / 7-13; one
    DMA + add merges them at the end, landing directly in conv3's rhs layout.
  BN+ReLU (ACT) -> conv3 1x1 (PE) -> BN (ACT) -> +residual (DVE) -> ReLU (ACT)
"""

import sys, types
sys.path.insert(0, "/opt/trn_rl_repo")

import numpy as np
import ml_dtypes
from contextlib import ExitStack

import concourse.bass as bass
import concourse.mybir as mybir
import concourse.tile as tile
from concourse import bacc
from concourse.bass import ts
from concourse.bass_utils import run_bass_kernel_spmd

BF16 = mybir.dt.bfloat16
F32 = mybir.dt.float32
AF = mybir.ActivationFunctionType

K = 15
G = 4
GC = 16
PAD = 7
CIN = 256
CMID = 64
RED = 16
COUT = 256
H = 56
W = 56
B = 2
HB = 14            # output rows per core
HP = HB + 2 * PAD  # 28 padded rows
WP = W + 2 * PAD   # 70 padded cols
NP = HP * WP       # 1960
HH = HB // 2       # 7 rows per half-block
NF = HH * W        # 392 pixels per half-block
NPAIR = 7 * K + 8  # 105 (ky,ky+7) pairs + 8 row-14 (kx,kx+1) pairs = 113
NQ = (NPAIR + 1) // 2  # 57 groups (2 pairs each; last group has 1 pair)
WCH = 38           # wse pairs per 32-partition chunk

_PROGRAM = None  # (nc, names) cache


def _build_program():
    nc = bacc.Bacc(None, target_bir_lowering=False, debug=False)
    with tile.TileContext(nc) as tc, ExitStack() as ctx:
        dram = ctx.enter_context(tc.tile_pool(name="dram", bufs=1, space="DRAM"))
        xb_d = dram.tile([CIN, NP], BF16, kind="ExternalInput", name="xb")
        xr_d = dram.tile([COUT, HB * W], F32, kind="ExternalInput", name="xr")
        w1t_d = dram.tile([CIN, CMID], BF16, kind="ExternalInput", name="w1t")
        wrt_d = dram.tile([CMID, RED], BF16, kind="ExternalInput", name="wrt")
        wse_d = dram.tile([81, WCH * 128], BF16, kind="ExternalInput", name="wse")
        w3t_d = dram.tile([CMID, COUT], BF16, kind="ExternalInput", name="w3t")
        vec_d = dram.tile([128, 10], F32, kind="ExternalInput", name="vecs")
        ones_d = dram.tile([1, 2 * NF], BF16, kind="ExternalInput", name="ones")
        y_d = dram.tile([COUT, HB * W], F32, kind="ExternalOutput", name="y")

        xpool = ctx.enter_context(tc.tile_pool(name="xin", bufs=1))
        xb = xpool.tile([128, 2, NP], BF16)
        xb_src = xb_d[:].rearrange("(c p) n -> p c n", p=128)
        nc.sync.dma_start(out=xb[:, :, 0:980], in_=xb_src[:, :, 0:980])
        nc.sync.dma_start(out=xb[:, :, 980:NP], in_=xb_src[:, :, 980:NP])

        wpool = ctx.enter_context(tc.tile_pool(name="weights", bufs=1))
        w1t = wpool.tile([128, 2, CMID], BF16)
        nc.sync.dma_start(out=w1t[:], in_=w1t_d[:].rearrange("(c p) m -> p c m", p=128))
        wrt = wpool.tile([CMID, RED], BF16)
        nc.sync.dma_start(out=wrt[:], in_=wrt_d[:])
        # span weights in 4 vertical chunks at partitions 0/32/64/96 so the
        # DMA spans 113 partitions and fans out across the HWDGE queues
        wse = wpool.tile([81, WCH * 128], BF16)
        nc.sync.dma_start(out=wse[:], in_=wse_d[:])
        w3t = wpool.tile([CMID, COUT], BF16)
        nc.sync.dma_start(out=w3t[:], in_=w3t_d[:])
        vecs = wpool.tile([128, 10], F32)
        nc.sync.dma_start(out=vecs[:], in_=vec_d[:])

        xr = xpool.tile([128, 2, HB * W], F32)
        nc.sync.dma_start(out=xr[:], in_=xr_d[:].rearrange("(c p) n -> p c n", p=128))

        opool = ctx.enter_context(tc.tile_pool(name="out1", bufs=1))
        out1p = opool.tile([128, NP], BF16)
        out1q = opool.tile([128, NP], BF16)
        out1r = opool.tile([128, NP], BF16)  # row-14 pairs: [0:64]=blockA, [64:128]=blockA<<1col

        # conv1: out1 = relu(g1 * (W1 @ x) + b1) over all 28x70 padded pixels
        with tc.tile_pool(name="p1", bufs=4, space="PSUM") as p1:
            for j in (1, 2, 3, 0):
                ps = p1.tile([CMID, 490], F32, tag="ps1")
                nc.tensor.matmul(ps[:], w1t[:, 0, :], xb[:, 0, ts(j, 490)],
                                 start=True, stop=False)
                nc.tensor.matmul(ps[:], w1t[:, 1, :], xb[:, 1, ts(j, 490)],
                                 start=False, stop=True)
                nc.scalar.activation(out1p[0:CMID, ts(j, 490)], ps[:], AF.Relu,
                                     bias=vecs[0:CMID, 1:2], scale=vecs[0:CMID, 0:1])

        # rows 7..27 duplicated at partitions 64:128 (the +7-row tap shift);
        # zero the unwritten tail there so zero-padded taps read 0, not junk.
        DUPW = (HP - HH) * WP  # 1470
        nc.vector.memset(out1p[CMID:128, DUPW:NP], 0.0)
        nc.vector.memset(out1q[CMID:128, DUPW - 1:NP], 0.0)
        nc.vector.memset(out1q[0:CMID, NP - 1:NP], 0.0)
        nc.sync.dma_start(out=out1p[CMID:128, 0:DUPW],
                          in_=out1p[0:CMID, HH * WP:NP])
        o3 = out1p[:].rearrange("p (h w) -> p h w", w=WP)
        o3q = out1q[:].rearrange("p (h w) -> p h w", w=WP)
        o3r = out1r[:].rearrange("p (h w) -> p h w", w=WP)

        spool = ctx.enter_context(tc.tile_pool(name="stage", bufs=1))
        r_sb = spool.tile([81, 2 * NF], BF16)
        # ones row (span bias via the K dimension); DMA since engines cannot
        # address a single partition at offset 16
        nc.sync.dma_start(out=r_sb[RED:RED + 1, :], in_=ones_d[:])

        # reduce: r = relu(gr * (Wr @ out1_central) + br), central 14x56 pixels
        with tc.tile_pool(name="pr", bufs=2, space="PSUM") as pr:
            for hhalf in range(2):
                ps = pr.tile([RED, NF], F32, tag="psr")
                nc.tensor.matmul(ps[:], wrt[:],
                                 o3[0:CMID, PAD + HH * hhalf:PAD + HH * (hhalf + 1), PAD:PAD + W],
                                 start=True, stop=True)
                nc.scalar.activation(r_sb[0:RED, ts(hhalf, NF)], ps[:], AF.Relu,
                                     bias=vecs[0:RED, 3:4], scale=vecs[0:RED, 2:3])

        # replicate r (+ones row) at partitions 32/64/96 to match the span
        # lhsT chunks' base partitions
        for c in range(1, 3):
            nc.sync.dma_start(out=r_sb[32 * c:32 * c + RED + 1, :],
                              in_=r_sb[0:RED + 1, :])

        # +1-col shifted copies (odd-kx 4B alignment) are same-partition, so
        # they run on ACT; placed after the reduce so r is produced first
        nc.vector.tensor_copy(out1q[0:CMID, 0:NP - 1], out1p[0:CMID, 1:NP])
        nc.vector.tensor_copy(out1q[CMID:128, 0:DUPW - 1], out1p[CMID:128, 1:DUPW])
        # row-14 pair source: partitions 64:128 hold block A shifted +1 col
        nc.vector.memset(out1r[CMID:128, NP - 1:NP], 0.0)
        nc.vector.tensor_copy(out1r[0:CMID, :], out1p[0:CMID, :])
        nc.sync.dma_start(out=out1r[CMID:128, 0:NP - 1], in_=out1p[0:CMID, 1:NP])

        # involution: 60 quad groups x (2 pairs x 2 pixel-halves)
        acc = None
        with tc.tile_pool(name="sp", bufs=2, space="PSUM") as sp, \
             tc.tile_pool(name="we", bufs=5) as we_pool, \
             tc.tile_pool(name="prod", bufs=4) as prod_pool, \
             tc.tile_pool(name="accp", bufs=4) as acc_pool:
            for q in range(NQ):
                npr = 2 if 2 * q + 1 < NPAIR else 1
                ps = sp.tile([128, 2048], F32, tag="spanps")
                for i in range(npr):
                    pi = 2 * q + i
                    wc, wo = pi // WCH, pi % WCH
                    lhsT = wse[32 * wc:32 * wc + RED + 1, ts(wo, 128)]
                    rr = r_sb[32 * wc:32 * wc + RED + 1, :]
                    nc.tensor.matmul(ps[:, i * 1024 + 0:i * 1024 + NF],
                                     lhsT, rr[:, 0:NF], start=True, stop=True)
                    nc.tensor.matmul(ps[:, i * 1024 + 512:i * 1024 + 512 + NF],
                                     lhsT, rr[:, NF:2 * NF], start=True, stop=True)
                we = we_pool.tile([128, 2 * npr, NF], BF16, tag="we")
                nc.scalar.activation(
                    we[:], ps[:].rearrange("p (q x) -> p q x", x=512)[:, 0:2 * npr, 0:NF],
                    AF.Copy, scale=1.0)
                prod = prod_pool.tile([128, 2, 2 * NF], BF16, tag="prod")
                for i in range(npr):
                    pi = 2 * q + i
                    if pi < 7 * K:
                        ky, kx = pi // K, pi % K
                        if kx % 2 == 0:
                            src_ = o3[:, ky:ky + 2 * HH, kx:kx + W]
                        else:
                            src_ = o3q[:, ky:ky + 2 * HH, kx - 1:kx - 1 + W]
                    else:
                        kx = 2 * (pi - 7 * K)  # row-14 pair (14,kx)+(14,kx+1)
                        src_ = o3r[:, 14:14 + 2 * HH, kx:kx + W]
                    nc.vector.tensor_mul(
                        prod[:, i, :].rearrange("p (b h w) -> p b h w", b=2, w=W),
                        we[:, 2 * i:2 * i + 2, :].rearrange("p b (h w) -> p b h w", w=W),
                        src_.rearrange("p (b h) w -> p b h w", b=2))
                if npr == 2:
                    s = acc_pool.tile([128, 2 * NF], BF16, tag="s")
                    nc.vector.tensor_add(s[:], prod[:, 0, :], prod[:, 1, :])
                else:
                    s = prod[:, 0, :]
                if acc is None:
                    acc = s
                else:
                    na = acc_pool.tile([128, 2 * NF], BF16, tag="acc")
                    nc.vector.tensor_add(na[:], acc[:], s[:])
                    acc = na

        # merge ky-groups: inv[c] = acc[c] + acc[64+c], pipelined by halves,
        # landing directly in conv3's rhs layout [64, 784]
        tmp = spool.tile([CMID, 2 * NF], BF16)
        inv = spool.tile([CMID, 2 * NF], BF16)
        out2f = spool.tile([CMID, 2 * NF], BF16)
        for h in range(2):
            nc.sync.dma_start(out=tmp[:, ts(h, NF)], in_=acc[CMID:128, ts(h, NF)])
            nc.vector.tensor_add(inv[:, ts(h, NF)], acc[0:CMID, ts(h, NF)],
                                 tmp[:, ts(h, NF)])
            nc.scalar.activation(out2f[:, ts(h, NF)], inv[:, ts(h, NF)], AF.Relu,
                                 bias=vecs[0:CMID, 5:6], scale=vecs[0:CMID, 4:5])

        # conv3 + BN3 + residual + relu
        with tc.tile_pool(name="p3", bufs=2, space="PSUM") as p3, \
             tc.tile_pool(name="ypool", bufs=2) as ypool:
            for nh in range(2):
                for mc in range(2):
                    ps = p3.tile([128, NF], F32, tag="ps3")
                    nc.tensor.matmul(ps[:], w3t[:, ts(mc, 128)], out2f[:, ts(nh, NF)],
                                     start=True, stop=True)
                    t3 = ypool.tile([128, NF], F32, tag="t3")
                    nc.scalar.activation(t3[:], ps[:], AF.Identity,
                                         bias=vecs[:, 8 + mc:9 + mc],
                                         scale=vecs[:, 6 + mc:7 + mc])
                    ys = ypool.tile([128, NF], F32, tag="ys")
                    nc.vector.tensor_add(ys[:], t3[:], xr[:, mc, ts(nh, NF)])
                    yr = ypool.tile([128, NF], F32, tag="yr")
                    nc.scalar.activation(yr[:], ys[:], AF.Relu, scale=1.0)
                    nc.sync.dma_start(
                        out=y_d[:].rearrange("(c p) n -> p c n", p=128)[:, mc, ts(nh, NF)],
                        in_=yr[:])

    nc.compile()
    names = dict(xb=xb_d.name, xr=xr_d.name, w1t=w1t_d.name, wrt=wrt_d.name,
                 wse=wse_d.name, w3t=w3t_d.name, vecs=vec_d.name,
                 ones=ones_d.name, y=y_d.name)
    return nc, names


def _get_program():
    global _PROGRAM
    if _PROGRAM is None:
        _PROGRAM = _build_program()
    return _PROGRAM


def _bf16(a):
    return np.asarray(a, dtype=np.float32).astype(ml_dtypes.bfloat16)


def kernel(x, W1, g1, b1, Wr, gr, br, Ws, bs, g2, b2, W3, g3, b3,
           _want_results=False, _trace=False):
    x = np.asarray(x, dtype=np.float32)
    nc, names = _get_program()

    w1t = _bf16(np.asarray(W1).T)                      # [256, 64]
    wrt = _bf16(np.asarray(Wr).T)                      # [64, 16]
    w3t = _bf16(np.asarray(W3).T)                      # [64, 256]

    # span weights, 16x channel-expanded, tap-paired (ky, ky+7), bias row 16.
    # wse[:, pi*128 + j]: j<64 -> tap (ky,kx), j>=64 -> tap (ky+7,kx) (zeros
    # for the ky=14 solo row).  pi = ky*15 + kx, ky in 0..7.
    Ws = np.asarray(Ws, dtype=np.float32)              # [900, 16]
    bs = np.asarray(bs, dtype=np.float32)              # [900]
    gidx = np.arange(CMID) // GC                       # [64]
    WsT = Ws.reshape(G, K * K, RED)                    # [g, k, rho]
    bsr = bs.reshape(G, K * K)
    wse = np.zeros((RED + 1, NPAIR, 128), dtype=np.float32)
    for pi in range(NPAIR):
        if pi < 7 * K:
            ky, kx = pi // K, pi % K
            k1, k2 = ky * K + kx, (ky + 7) * K + kx
        else:
            kx = 2 * (pi - 7 * K)
            k1 = 14 * K + kx
            k2 = 14 * K + kx + 1 if kx + 1 < K else None
        wse[0:RED, pi, 0:CMID] = WsT[gidx, k1, :].T
        wse[RED, pi, 0:CMID] = bsr[gidx, k1]
        if k2 is not None:
            wse[0:RED, pi, CMID:128] = WsT[gidx, k2, :].T
            wse[RED, pi, CMID:128] = bsr[gidx, k2]
    wse4 = np.zeros((81, WCH * 128), dtype=np.float32)
    for pi in range(NPAIR):
        wc, wo = pi // WCH, pi % WCH
        wse4[32 * wc:32 * wc + RED + 1, wo * 128:(wo + 1) * 128] = wse[:, pi, :]
    wse = _bf16(wse4)

    vecs = np.zeros((128, 10), dtype=np.float32)
    vecs[0:CMID, 0] = g1
    vecs[0:CMID, 1] = b1
    vecs[0:RED, 2] = gr
    vecs[0:RED, 3] = br
    vecs[0:CMID, 4] = g2
    vecs[0:CMID, 5] = b2
    vecs[:, 6] = np.asarray(g3)[0:128]
    vecs[:, 7] = np.asarray(g3)[128:256]
    vecs[:, 8] = np.asarray(b3)[0:128]
    vecs[:, 9] = np.asarray(b3)[128:256]

    in_maps = []
    core_geom = []
    for core in range(8):
        b = core // 4
        h0 = (core % 4) * HB
        xpad = np.zeros((CIN, HP, WP), dtype=np.float32)
        lo, hi = h0 - PAD, h0 + HB + PAD
        slo, shi = max(lo, 0), min(hi, H)
        xpad[:, slo - lo:shi - lo, PAD:PAD + W] = x[b, :, slo:shi, :]
        xbc = _bf16(xpad).reshape(CIN, NP)
        xrc = np.ascontiguousarray(x[b, :, h0:h0 + HB, :]).reshape(COUT, HB * W)
        in_maps.append({
            names["xb"]: xbc,
            names["xr"]: xrc,
            names["w1t"]: w1t,
            names["wrt"]: wrt,
            names["wse"]: wse,
            names["w3t"]: w3t,
            names["vecs"]: vecs,
            names["ones"]: np.ones((1, 2 * NF), dtype=np.float32).astype(ml_dtypes.bfloat16),
        })
        core_geom.append((b, h0))

    res = run_bass_kernel_spmd(nc, in_maps, list(range(8)), trace=_trace)

    y = np.empty((B, COUT, H, W), dtype=np.float32)
    for core, (b, h0) in enumerate(core_geom):
        y[b, :, h0:h0 + HB, :] = res.results[core][names["y"]].reshape(COUT, HB, W)
    if _want_results:
        return y, res
    return y


# revision 45
# speedup vs baseline: 1.0085x; 1.0009x over previous
"""Trainium2 Bass kernel for an involution Bottleneck block (B=2, Cin=256,
Cmid=64, Cout=256, H=W=56, K=15, G=4).

Sharding: 8 cores = 2 batches x 4 H-quarters (14 output rows each). Each core
receives a zero-padded input halo [256, 28, 70] (7 rows/cols each side), so no
inter-core communication is needed (halo compute is redundant).

Per-core pipeline (channels on SBUF partitions, pixels on free dim):
  conv1 1x1 (PE, bf16) -> BN+ReLU (ACT) -> out1 [64, 28x70] bf16, plus a copy
    shifted by +7 rows at partitions 64:128 and a +1-col copy (odd-kx align).
  reduce 1x1 (PE) -> BN+ReLU (ACT) -> r [16, 784] bf16 (+ ones row for bias).
  involution as 120 tap-PAIRS (ky, ky+7): one span matmul with M=128 columns
    (64 per tap, Ws columns 16x-expanded, bias via the ones-row at K=17)
    yields the per-pixel kernels for both taps stacked on partitions; ACT
    evicts 4 PSUM banks (2 pairs x 2 pixel-halves) per op; DVE multiplies by
    the shifted out1 window with a 4D AP covering both pixel halves (bf16 2x)
    and accumulates. Partition halves hold ky-groups 0-6 / 7-13; one
    DMA + add merges them at the end, landing directly in conv3's rhs layout.
  BN+ReLU (ACT) -> conv3 1x1 (PE) -> BN (ACT) -> +residual (DVE) -> ReLU (ACT)
"""

import sys, types
sys.path.insert(0, "/opt/trn_rl_repo")

import numpy as np
import ml_dtypes
from contextlib import ExitStack

import concourse.bass as bass
import concourse.mybir as mybir
import concourse.tile as tile
from concourse import bacc
from concourse.bass import ts
from concourse.bass_utils import run_bass_kernel_spmd

BF16 = mybir.dt.bfloat16
F32 = mybir.dt.float32
AF = mybir.ActivationFunctionType

K = 15
G = 4
GC = 16
PAD = 7
CIN = 256
CMID = 64
RED = 16
COUT = 256
H = 56
W = 56
B = 2
HB = 14            # output rows per core
HP = HB + 2 * PAD  # 28 padded rows
WP = W + 2 * PAD   # 70 padded cols
NP = HP * WP       # 1960
HH = HB // 2       # 7 rows per half-block
NF = HH * W        # 392 pixels per half-block
NPAIR = 7 * K + 8  # 105 (ky,ky+7) pairs + 8 row-14 (kx,kx+1) pairs = 113
NQ = (NPAIR + 1) // 2  # 57 groups (2 pairs each; last group has 1 pair)
WCH = 38           # wse pairs per 32-partition chunk

_PROGRAM = None  # (nc, names) cache


def _build_program():
    nc = bacc.Bacc(None, target_bir_lowering=False, debug=False)
    with tile.TileContext(nc) as tc, ExitStack() as ctx:
        dram = ctx.enter_context(tc.tile_pool(name="dram", bufs=1, space="DRAM"))
        xb_d = dram.tile([CIN, NP], BF16, kind="ExternalInput", name="xb")
        xr_d = dram.tile([COUT, HB * W], F32, kind="ExternalInput", name="xr")
        w1t_d = dram.tile([CIN, CMID], BF16, kind="ExternalInput", name="w1t")
        wrt_d = dram.tile([CMID, RED], BF16, kind="ExternalInput", name="wrt")
        wse_d = dram.tile([81, WCH * 128], BF16, kind="ExternalInput", name="wse")
        w3t_d = dram.tile([CMID, COUT], BF16, kind="ExternalInput", name="w3t")
        vec_d = dram.tile([128, 10], F32, kind="ExternalInput", name="vecs")
        ones_d = dram.tile([1, 2 * NF], BF16, kind="ExternalInput", name="ones")
        y_d = dram.tile([COUT, HB * W], F32, kind="ExternalOutput", name="y")

        xpool = ctx.enter_context(tc.tile_pool(name="xin", bufs=1))
        xb = xpool.tile([128, 2, NP], BF16)
        xb_src = xb_d[:].rearrange("(c p) n -> p c n", p=128)
        nc.sync.dma_start(out=xb[:, :, 0:980], in_=xb_src[:, :, 0:980])
        nc.sync.dma_start(out=xb[:, :, 980:NP], in_=xb_src[:, :, 980:NP])

        wpool = ctx.enter_context(tc.tile_pool(name="weights", bufs=1))
        w1t = wpool.tile([128, 2, CMID], BF16)
        nc.sync.dma_start(out=w1t[:], in_=w1t_d[:].rearrange("(c p) m -> p c m", p=128))
        wrt = wpool.tile([CMID, RED], BF16)
        nc.sync.dma_start(out=wrt[:], in_=wrt_d[:])
        # span weights in 4 vertical chunks at partitions 0/32/64/96 so the
        # DMA spans 113 partitions and fans out across the HWDGE queues
        wse = wpool.tile([81, WCH * 128], BF16)
        nc.sync.dma_start(out=wse[:], in_=wse_d[:])
        w3t = wpool.tile([CMID, COUT], BF16)
        nc.sync.dma_start(out=w3t[:], in_=w3t_d[:])
        vecs = wpool.tile([128, 10], F32)
        nc.sync.dma_start(out=vecs[:], in_=vec_d[:])

        xr = xpool.tile([128, 2, HB * W], F32)
        nc.sync.dma_start(out=xr[:], in_=xr_d[:].rearrange("(c p) n -> p c n", p=128))

        opool = ctx.enter_context(tc.tile_pool(name="out1", bufs=1))
        out1p = opool.tile([128, NP], BF16)
        out1q = opool.tile([128, NP], BF16)
        out1r = opool.tile([128, NP], BF16)  # row-14 pairs: [0:64]=blockA, [64:128]=blockA<<1col

        # conv1: out1 = relu(g1 * (W1 @ x) + b1) over all 28x70 padded pixels
        with tc.tile_pool(name="p1", bufs=4, space="PSUM") as p1:
            for j in (1, 2, 3, 0):
                ps = p1.tile([CMID, 490], F32, tag="ps1")
                nc.tensor.matmul(ps[:], w1t[:, 0, :], xb[:, 0, ts(j, 490)],
                                 start=True, stop=False)
                nc.tensor.matmul(ps[:], w1t[:, 1, :], xb[:, 1, ts(j, 490)],
                                 start=False, stop=True)
                nc.scalar.activation(out1p[0:CMID, ts(j, 490)], ps[:], AF.Relu,
                                     bias=vecs[0:CMID, 1:2], scale=vecs[0:CMID, 0:1])

        # rows 7..27 duplicated at partitions 64:128 (the +7-row tap shift);
        # zero the unwritten tail there so zero-padded taps read 0, not junk.
        DUPW = (HP - HH) * WP  # 1470
        nc.vector.memset(out1p[CMID:128, DUPW:NP], 0.0)
        nc.vector.memset(out1q[CMID:128, DUPW - 1:NP], 0.0)
        nc.vector.memset(out1q[0:CMID, NP - 1:NP], 0.0)
        nc.sync.dma_start(out=out1p[CMID:128, 0:DUPW],
                          in_=out1p[0:CMID, HH * WP:NP])
        o3 = out1p[:].rearrange("p (h w) -> p h w", w=WP)
        o3q = out1q[:].rearrange("p (h w) -> p h w", w=WP)
        o3r = out1r[:].rearrange("p (h w) -> p h w", w=WP)

        spool = ctx.enter_context(tc.tile_pool(name="stage", bufs=1))
        r_sb = spool.tile([81, 2 * NF], BF16)
        # ones row (span bias via the K dimension); DMA since engines cannot
        # address a single partition at offset 16
        nc.sync.dma_start(out=r_sb[RED:RED + 1, :], in_=ones_d[:])

        # reduce: r = relu(gr * (Wr @ out1_central) + br), central 14x56 pixels
        with tc.tile_pool(name="pr", bufs=2, space="PSUM") as pr:
            for hhalf in range(2):
                ps = pr.tile([RED, NF], F32, tag="psr")
                nc.tensor.matmul(ps[:], wrt[:],
                                 o3[0:CMID, PAD + HH * hhalf:PAD + HH * (hhalf + 1), PAD:PAD + W],
                                 start=True, stop=True)
                nc.scalar.activation(r_sb[0:RED, ts(hhalf, NF)], ps[:], AF.Relu,
                                     bias=vecs[0:RED, 3:4], scale=vecs[0:RED, 2:3])

        # replicate r (+ones row) at partitions 32/64/96 to match the span
        # lhsT chunks' base partitions
        for c in range(1, 3):
            nc.sync.dma_start(out=r_sb[32 * c:32 * c + RED + 1, :],
                              in_=r_sb[0:RED + 1, :])

        # +1-col shifted copies (odd-kx 4B alignment) are same-partition, so
        # they run on ACT; placed after the reduce so r is produced first
        nc.vector.tensor_copy(out1q[0:CMID, 0:NP - 1], out1p[0:CMID, 1:NP])
        nc.vector.tensor_copy(out1q[CMID:128, 0:DUPW - 1], out1p[CMID:128, 1:DUPW])
        # row-14 pair source: partitions 64:128 hold block A shifted +1 col
        nc.vector.memset(out1r[CMID:128, NP - 1:NP], 0.0)
        nc.vector.tensor_copy(out1r[0:CMID, :], out1p[0:CMID, :])
        nc.sync.dma_start(out=out1r[CMID:128, 0:NP - 1], in_=out1p[0:CMID, 1:NP])

        # involution: 60 quad groups x (2 pairs x 2 pixel-halves)
        acc = None
        with tc.tile_pool(name="sp", bufs=2, space="PSUM") as sp, \
             tc.tile_pool(name="we", bufs=5) as we_pool, \
             tc.tile_pool(name="prod", bufs=4) as prod_pool, \
             tc.tile_pool(name="accp", bufs=4) as acc_pool:
            prod = None
            for q in range(NQ):
                npr = 2 if 2 * q + 1 < NPAIR else 1
                ps = sp.tile([128, 2048], F32, tag="spanps")
                for i in range(npr):
                    pi = 2 * q + i
                    wc, wo = pi // WCH, pi % WCH
                    lhsT = wse[32 * wc:32 * wc + RED + 1, ts(wo, 128)]
                    rr = r_sb[32 * wc:32 * wc + RED + 1, :]
                    nc.tensor.matmul(ps[:, i * 1024 + 0:i * 1024 + NF],
                                     lhsT, rr[:, 0:NF], start=True, stop=True)
                    nc.tensor.matmul(ps[:, i * 1024 + 512:i * 1024 + 512 + NF],
                                     lhsT, rr[:, NF:2 * NF], start=True, stop=True)
                we = we_pool.tile([128, 2 * npr, NF], BF16, tag="we")
                nc.scalar.activation(
                    we[:], ps[:].rearrange("p (q x) -> p q x", x=512)[:, 0:2 * npr, 0:NF],
                    AF.Copy, scale=1.0)
                if q % 2 == 0:
                    prod = prod_pool.tile([128, 4, 2 * NF], BF16, tag="prod")
                for i in range(npr):
                    pi = 2 * q + i
                    if pi < 7 * K:
                        ky, kx = pi // K, pi % K
                        if kx % 2 == 0:
                            src_ = o3[:, ky:ky + 2 * HH, kx:kx + W]
                        else:
                            src_ = o3q[:, ky:ky + 2 * HH, kx - 1:kx - 1 + W]
                    else:
                        kx = 2 * (pi - 7 * K)  # row-14 pair (14,kx)+(14,kx+1)
                        src_ = o3r[:, 14:14 + 2 * HH, kx:kx + W]
                    pl = 2 * (q % 2) + i
                    nc.vector.tensor_mul(
                        prod[:, pl, :].rearrange("p (b h w) -> p b h w", b=2, w=W),
                        we[:, 2 * i:2 * i + 2, :].rearrange("p b (h w) -> p b h w", w=W),
                        src_.rearrange("p (b h) w -> p b h w", b=2))
                # every second group: wide 2-in-1 add over 4 pair-planes
                if q % 2 == 1:
                    s1 = acc_pool.tile([128, 2, 2 * NF], BF16, tag="s1")
                    nc.vector.tensor_add(s1[:], prod[:, 0:2, :], prod[:, 2:4, :])
                    s = acc_pool.tile([128, 2 * NF], BF16, tag="s")
                    nc.vector.tensor_add(s[:], s1[:, 0, :], s1[:, 1, :])
                elif q == NQ - 1:  # trailing group (1 or 2 pairs)
                    if npr == 2:
                        s = acc_pool.tile([128, 2 * NF], BF16, tag="s")
                        nc.vector.tensor_add(s[:], prod[:, 0, :], prod[:, 1, :])
                    else:
                        s = prod[:, 0, :]
                else:
                    continue
                if acc is None:
                    acc = s
                else:
                    na = acc_pool.tile([128, 2 * NF], BF16, tag="acc")
                    nc.vector.tensor_add(na[:], acc[:], s[:])
                    acc = na

        # merge ky-groups: inv[c] = acc[c] + acc[64+c], pipelined by halves,
        # landing directly in conv3's rhs layout [64, 784]
        tmp = spool.tile([CMID, 2 * NF], BF16)
        inv = spool.tile([CMID, 2 * NF], BF16)
        out2f = spool.tile([CMID, 2 * NF], BF16)
        for h in range(2):
            nc.sync.dma_start(out=tmp[:, ts(h, NF)], in_=acc[CMID:128, ts(h, NF)])
            nc.vector.tensor_add(inv[:, ts(h, NF)], acc[0:CMID, ts(h, NF)],
                                 tmp[:, ts(h, NF)])
            nc.scalar.activation(out2f[:, ts(h, NF)], inv[:, ts(h, NF)], AF.Relu,
                                 bias=vecs[0:CMID, 5:6], scale=vecs[0:CMID, 4:5])

        # conv3 + BN3 + residual + relu
        with tc.tile_pool(name="p3", bufs=2, space="PSUM") as p3, \
             tc.tile_pool(name="ypool", bufs=2) as ypool:
            for nh in range(2):
                for mc in range(2):
                    ps = p3.tile([128, NF], F32, tag="ps3")
                    nc.tensor.matmul(ps[:], w3t[:, ts(mc, 128)], out2f[:, ts(nh, NF)],
                                     start=True, stop=True)
                    t3 = ypool.tile([128, NF], F32, tag="t3")
                    nc.scalar.activation(t3[:], ps[:], AF.Identity,
                                         bias=vecs[:, 8 + mc:9 + mc],
                                         scale=vecs[:, 6 + mc:7 + mc])
                    ys = ypool.tile([128, NF], F32, tag="ys")
                    nc.vector.tensor_add(ys[:], t3[:], xr[:, mc, ts(nh, NF)])
                    yr = ypool.tile([128, NF], F32, tag="yr")
                    nc.scalar.activation(yr[:], ys[:], AF.Relu, scale=1.0)
                    nc.sync.dma_start(
                        out=y_d[:].rearrange("(c p) n -> p c n", p=128)[:, mc, ts(nh, NF)],
                        in_=yr[:])

    nc.compile()
    names = dict(xb=xb_d.name, xr=xr_d.name, w1t=w1t_d.name, wrt=wrt_d.name,
                 wse=wse_d.name, w3t=w3t_d.name, vecs=vec_d.name,
                 ones=ones_d.name, y=y_d.name)
    return nc, names


def _get_program():
    global _PROGRAM
    if _PROGRAM is None:
        _PROGRAM = _build_program()
    return _PROGRAM


def _bf16(a):
    return np.asarray(a, dtype=np.float32).astype(ml_dtypes.bfloat16)


def kernel(x, W1, g1, b1, Wr, gr, br, Ws, bs, g2, b2, W3, g3, b3,
           _want_results=False, _trace=False):
    x = np.asarray(x, dtype=np.float32)
    nc, names = _get_program()

    w1t = _bf16(np.asarray(W1).T)                      # [256, 64]
    wrt = _bf16(np.asarray(Wr).T)                      # [64, 16]
    w3t = _bf16(np.asarray(W3).T)                      # [64, 256]

    # span weights, 16x channel-expanded, tap-paired (ky, ky+7), bias row 16.
    # wse[:, pi*128 + j]: j<64 -> tap (ky,kx), j>=64 -> tap (ky+7,kx) (zeros
    # for the ky=14 solo row).  pi = ky*15 + kx, ky in 0..7.
    Ws = np.asarray(Ws, dtype=np.float32)              # [900, 16]
    bs = np.asarray(bs, dtype=np.float32)              # [900]
    gidx = np.arange(CMID) // GC                       # [64]
    WsT = Ws.reshape(G, K * K, RED)                    # [g, k, rho]
    bsr = bs.reshape(G, K * K)
    wse = np.zeros((RED + 1, NPAIR, 128), dtype=np.float32)
    for pi in range(NPAIR):
        if pi < 7 * K:
            ky, kx = pi // K, pi % K
            k1, k2 = ky * K + kx, (ky + 7) * K + kx
        else:
            kx = 2 * (pi - 7 * K)
            k1 = 14 * K + kx
            k2 = 14 * K + kx + 1 if kx + 1 < K else None
        wse[0:RED, pi, 0:CMID] = WsT[gidx, k1, :].T
        wse[RED, pi, 0:CMID] = bsr[gidx, k1]
        if k2 is not None:
            wse[0:RED, pi, CMID:128] = WsT[gidx, k2, :].T
            wse[RED, pi, CMID:128] = bsr[gidx, k2]
    wse4 = np.zeros((81, WCH * 128), dtype=np.float32)
    for pi in range(NPAIR):
        wc, wo = pi // WCH, pi % WCH
        wse4[32 * wc:32 * wc + RED + 1, wo * 128:(wo + 1) * 128] = wse[:, pi, :]
    wse = _bf16(wse4)

    vecs = np.zeros((128, 10), dtype=np.float32)
    vecs[0:CMID, 0] = g1
    vecs[0:CMID, 1] = b1
    vecs[0:RED, 2] = gr
    vecs[0:RED, 3] = br
    vecs[0:CMID, 4] = g2
    vecs[0:CMID, 5] = b2
    vecs[:, 6] = np.asarray(g3)[0:128]
    vecs[:, 7] = np.asarray(g3)[128:256]
    vecs[:, 8] = np.asarray(b3)[0:128]
    vecs[:, 9] = np.asarray(b3)[128:256]

    in_maps = []
    core_geom = []
    for core in range(8):
        b = core // 4
        h0 = (core % 4) * HB
        xpad = np.zeros((CIN, HP, WP), dtype=np.float32)
        lo, hi = h0 - PAD, h0 + HB + PAD
        slo, shi = max(lo, 0), min(hi, H)
        xpad[:, slo - lo:shi - lo, PAD:PAD + W] = x[b, :, slo:shi, :]
        xbc = _bf16(xpad).reshape(CIN, NP)
        xrc = np.ascontiguousarray(x[b, :, h0:h0 + HB, :]).reshape(COUT, HB * W)
        in_maps.append({
            names["xb"]: xbc,
            names["xr"]: xrc,
            names["w1t"]: w1t,
            names["wrt"]: wrt,
            names["wse"]: wse,
            names["w3t"]: w3t,
            names["vecs"]: vecs,
            names["ones"]: np.ones((1, 2 * NF), dtype=np.float32).astype(ml_dtypes.bfloat16),
        })
        core_geom.append((b, h0))

    res = run_bass_kernel_spmd(nc, in_maps, list(range(8)), trace=_trace)

    y = np.empty((B, COUT, H, W), dtype=np.float32)
    for core, (b, h0) in enumerate(core_geom):
        y[b, :, h0:h0 + HB, :] = res.results[core][names["y"]].reshape(COUT, HB, W)
    if _want_results:
        return y, res
    return y


# revision 46
# speedup vs baseline: 1.0101x; 1.0015x over previous
"""Trainium2 Bass kernel for an involution Bottleneck block (B=2, Cin=256,
Cmid=64, Cout=256, H=W=56, K=15, G=4).

Sharding: 8 cores = 2 batches x 4 H-quarters (14 output rows each). Each core
receives a zero-padded input halo [256, 28, 70] (7 rows/cols each side), so no
inter-core communication is needed (halo compute is redundant).

Per-core pipeline (channels on SBUF partitions, pixels on free dim):
  conv1 1x1 (PE, bf16) -> BN+ReLU (ACT) -> out1 [64, 28x70] bf16, plus a copy
    shifted by +7 rows at partitions 64:128 and a +1-col copy (odd-kx align).
  reduce 1x1 (PE) -> BN+ReLU (ACT) -> r [16, 784] bf16 (+ ones row for bias).
  involution as 120 tap-PAIRS (ky, ky+7): one span matmul with M=128 columns
    (64 per tap, Ws columns 16x-expanded, bias via the ones-row at K=17)
    yields the per-pixel kernels for both taps stacked on partitions; ACT
    evicts 4 PSUM banks (2 pairs x 2 pixel-halves) per op; DVE multiplies by
    the shifted out1 window with a 4D AP covering both pixel halves (bf16 2x)
    and accumulates. Partition halves hold ky-groups 0-6 / 7-13; one
    DMA + add merges them at the end, landing directly in conv3's rhs layout.
  BN+ReLU (ACT) -> conv3 1x1 (PE) -> BN (ACT) -> +residual (DVE) -> ReLU (ACT)
"""

import sys, types
sys.path.insert(0, "/opt/trn_rl_repo")

import numpy as np
import ml_dtypes
from contextlib import ExitStack

import concourse.bass as bass
import concourse.mybir as mybir
import concourse.tile as tile
from concourse import bacc
from concourse.bass import ts
from concourse.bass_utils import run_bass_kernel_spmd

BF16 = mybir.dt.bfloat16
F32 = mybir.dt.float32
AF = mybir.ActivationFunctionType

K = 15
G = 4
GC = 16
PAD = 7
CIN = 256
CMID = 64
RED = 16
COUT = 256
H = 56
W = 56
B = 2
HB = 14            # output rows per core
HP = HB + 2 * PAD  # 28 padded rows
WP = W + 2 * PAD   # 70 padded cols
NP = HP * WP       # 1960
HH = HB // 2       # 7 rows per half-block
NF = HH * W        # 392 pixels per half-block
NPAIR = 7 * K + 8  # 105 (ky,ky+7) pairs + 8 row-14 (kx,kx+1) pairs = 113
NQ = (NPAIR + 1) // 2  # 57 groups (2 pairs each; last group has 1 pair)
WCH = 38           # wse pairs per 32-partition chunk

_PROGRAM = None  # (nc, names) cache


def _build_program():
    nc = bacc.Bacc(None, target_bir_lowering=False, debug=False)
    with tile.TileContext(nc) as tc, ExitStack() as ctx:
        dram = ctx.enter_context(tc.tile_pool(name="dram", bufs=1, space="DRAM"))
        xb_d = dram.tile([CIN, NP], BF16, kind="ExternalInput", name="xb")
        xr_d = dram.tile([COUT, HB * W], BF16, kind="ExternalInput", name="xr")
        w1t_d = dram.tile([CIN, CMID], BF16, kind="ExternalInput", name="w1t")
        wrt_d = dram.tile([CMID, RED], BF16, kind="ExternalInput", name="wrt")
        wse_d = dram.tile([81, WCH * 128], BF16, kind="ExternalInput", name="wse")
        w3t_d = dram.tile([CMID, COUT], BF16, kind="ExternalInput", name="w3t")
        vec_d = dram.tile([128, 10], F32, kind="ExternalInput", name="vecs")
        ones_d = dram.tile([1, 2 * NF], BF16, kind="ExternalInput", name="ones")
        y_d = dram.tile([COUT, HB * W], F32, kind="ExternalOutput", name="y")

        xpool = ctx.enter_context(tc.tile_pool(name="xin", bufs=1))
        xb = xpool.tile([128, 2, NP], BF16)
        xb_src = xb_d[:].rearrange("(c p) n -> p c n", p=128)
        nc.sync.dma_start(out=xb[:, :, 0:980], in_=xb_src[:, :, 0:980])
        nc.sync.dma_start(out=xb[:, :, 980:NP], in_=xb_src[:, :, 980:NP])

        wpool = ctx.enter_context(tc.tile_pool(name="weights", bufs=1))
        w1t = wpool.tile([128, 2, CMID], BF16)
        nc.sync.dma_start(out=w1t[:], in_=w1t_d[:].rearrange("(c p) m -> p c m", p=128))
        wrt = wpool.tile([CMID, RED], BF16)
        nc.sync.dma_start(out=wrt[:], in_=wrt_d[:])
        # span weights in 4 vertical chunks at partitions 0/32/64/96 so the
        # DMA spans 113 partitions and fans out across the HWDGE queues
        wse = wpool.tile([81, WCH * 128], BF16)
        nc.sync.dma_start(out=wse[:], in_=wse_d[:])
        w3t = wpool.tile([CMID, COUT], BF16)
        nc.sync.dma_start(out=w3t[:], in_=w3t_d[:])
        vecs = wpool.tile([128, 10], F32)
        nc.sync.dma_start(out=vecs[:], in_=vec_d[:])

        xr = xpool.tile([128, 2, HB * W], BF16)
        nc.sync.dma_start(out=xr[:], in_=xr_d[:].rearrange("(c p) n -> p c n", p=128))

        opool = ctx.enter_context(tc.tile_pool(name="out1", bufs=1))
        out1p = opool.tile([128, NP], BF16)
        out1q = opool.tile([128, NP], BF16)
        out1r = opool.tile([128, NP], BF16)  # row-14 pairs: [0:64]=blockA, [64:128]=blockA<<1col

        # conv1: out1 = relu(g1 * (W1 @ x) + b1) over all 28x70 padded pixels
        with tc.tile_pool(name="p1", bufs=4, space="PSUM") as p1:
            for j in (1, 2, 3, 0):
                ps = p1.tile([CMID, 490], F32, tag="ps1")
                nc.tensor.matmul(ps[:], w1t[:, 0, :], xb[:, 0, ts(j, 490)],
                                 start=True, stop=False)
                nc.tensor.matmul(ps[:], w1t[:, 1, :], xb[:, 1, ts(j, 490)],
                                 start=False, stop=True)
                nc.scalar.activation(out1p[0:CMID, ts(j, 490)], ps[:], AF.Relu,
                                     bias=vecs[0:CMID, 1:2], scale=vecs[0:CMID, 0:1])

        # rows 7..27 duplicated at partitions 64:128 (the +7-row tap shift);
        # zero the unwritten tail there so zero-padded taps read 0, not junk.
        DUPW = (HP - HH) * WP  # 1470
        nc.vector.memset(out1p[CMID:128, DUPW:NP], 0.0)
        nc.vector.memset(out1q[CMID:128, DUPW - 1:NP], 0.0)
        nc.vector.memset(out1q[0:CMID, NP - 1:NP], 0.0)
        nc.sync.dma_start(out=out1p[CMID:128, 0:DUPW],
                          in_=out1p[0:CMID, HH * WP:NP])
        o3 = out1p[:].rearrange("p (h w) -> p h w", w=WP)
        o3q = out1q[:].rearrange("p (h w) -> p h w", w=WP)
        o3r = out1r[:].rearrange("p (h w) -> p h w", w=WP)

        spool = ctx.enter_context(tc.tile_pool(name="stage", bufs=1))
        r_sb = spool.tile([81, 2 * NF], BF16)
        # ones row (span bias via the K dimension); DMA since engines cannot
        # address a single partition at offset 16
        nc.sync.dma_start(out=r_sb[RED:RED + 1, :], in_=ones_d[:])

        # reduce: r = relu(gr * (Wr @ out1_central) + br), central 14x56 pixels
        with tc.tile_pool(name="pr", bufs=2, space="PSUM") as pr:
            for hhalf in range(2):
                ps = pr.tile([RED, NF], F32, tag="psr")
                nc.tensor.matmul(ps[:], wrt[:],
                                 o3[0:CMID, PAD + HH * hhalf:PAD + HH * (hhalf + 1), PAD:PAD + W],
                                 start=True, stop=True)
                nc.scalar.activation(r_sb[0:RED, ts(hhalf, NF)], ps[:], AF.Relu,
                                     bias=vecs[0:RED, 3:4], scale=vecs[0:RED, 2:3])

        # replicate r (+ones row) at partitions 32/64/96 to match the span
        # lhsT chunks' base partitions
        for c in range(1, 3):
            nc.sync.dma_start(out=r_sb[32 * c:32 * c + RED + 1, :],
                              in_=r_sb[0:RED + 1, :])

        # +1-col shifted copies (odd-kx 4B alignment) are same-partition, so
        # they run on ACT; placed after the reduce so r is produced first
        nc.vector.tensor_copy(out1q[0:CMID, 0:NP - 1], out1p[0:CMID, 1:NP])
        nc.vector.tensor_copy(out1q[CMID:128, 0:DUPW - 1], out1p[CMID:128, 1:DUPW])
        # row-14 pair source: partitions 64:128 hold block A shifted +1 col
        nc.vector.memset(out1r[CMID:128, NP - 1:NP], 0.0)
        nc.vector.tensor_copy(out1r[0:CMID, :], out1p[0:CMID, :])
        nc.sync.dma_start(out=out1r[CMID:128, 0:NP - 1], in_=out1p[0:CMID, 1:NP])

        # involution: 60 quad groups x (2 pairs x 2 pixel-halves)
        acc = None
        with tc.tile_pool(name="sp", bufs=2, space="PSUM") as sp, \
             tc.tile_pool(name="we", bufs=5) as we_pool, \
             tc.tile_pool(name="prod", bufs=4) as prod_pool, \
             tc.tile_pool(name="accp", bufs=4) as acc_pool:
            prod = None
            for q in range(NQ):
                npr = 2 if 2 * q + 1 < NPAIR else 1
                ps = sp.tile([128, 2048], F32, tag="spanps")
                for i in range(npr):
                    pi = 2 * q + i
                    wc, wo = pi // WCH, pi % WCH
                    lhsT = wse[32 * wc:32 * wc + RED + 1, ts(wo, 128)]
                    rr = r_sb[32 * wc:32 * wc + RED + 1, :]
                    nc.tensor.matmul(ps[:, i * 1024 + 0:i * 1024 + NF],
                                     lhsT, rr[:, 0:NF], start=True, stop=True)
                    nc.tensor.matmul(ps[:, i * 1024 + 512:i * 1024 + 512 + NF],
                                     lhsT, rr[:, NF:2 * NF], start=True, stop=True)
                we = we_pool.tile([128, 2 * npr, NF], BF16, tag="we")
                nc.scalar.activation(
                    we[:], ps[:].rearrange("p (q x) -> p q x", x=512)[:, 0:2 * npr, 0:NF],
                    AF.Copy, scale=1.0)
                if q % 2 == 0:
                    prod = prod_pool.tile([128, 4, 2 * NF], BF16, tag="prod")
                for i in range(npr):
                    pi = 2 * q + i
                    if pi < 7 * K:
                        ky, kx = pi // K, pi % K
                        if kx % 2 == 0:
                            src_ = o3[:, ky:ky + 2 * HH, kx:kx + W]
                        else:
                            src_ = o3q[:, ky:ky + 2 * HH, kx - 1:kx - 1 + W]
                    else:
                        kx = 2 * (pi - 7 * K)  # row-14 pair (14,kx)+(14,kx+1)
                        src_ = o3r[:, 14:14 + 2 * HH, kx:kx + W]
                    pl = 2 * (q % 2) + i
                    nc.vector.tensor_mul(
                        prod[:, pl, :].rearrange("p (b h w) -> p b h w", b=2, w=W),
                        we[:, 2 * i:2 * i + 2, :].rearrange("p b (h w) -> p b h w", w=W),
                        src_.rearrange("p (b h) w -> p b h w", b=2))
                # every second group: wide 2-in-1 add over 4 pair-planes
                if q % 2 == 1:
                    s1 = acc_pool.tile([128, 2, 2 * NF], BF16, tag="s1")
                    nc.vector.tensor_add(s1[:], prod[:, 0:2, :], prod[:, 2:4, :])
                    s = acc_pool.tile([128, 2 * NF], BF16, tag="s")
                    nc.vector.tensor_add(s[:], s1[:, 0, :], s1[:, 1, :])
                elif q == NQ - 1:  # trailing group (1 or 2 pairs)
                    if npr == 2:
                        s = acc_pool.tile([128, 2 * NF], BF16, tag="s")
                        nc.vector.tensor_add(s[:], prod[:, 0, :], prod[:, 1, :])
                    else:
                        s = prod[:, 0, :]
                else:
                    continue
                if acc is None:
                    acc = s
                else:
                    na = acc_pool.tile([128, 2 * NF], BF16, tag="acc")
                    nc.vector.tensor_add(na[:], acc[:], s[:])
                    acc = na

        # merge ky-groups: inv[c] = acc[c] + acc[64+c], pipelined by halves,
        # landing directly in conv3's rhs layout [64, 784]
        tmp = spool.tile([CMID, 2 * NF], BF16)
        inv = spool.tile([CMID, 2 * NF], BF16)
        out2f = spool.tile([CMID, 2 * NF], BF16)
        for h in range(2):
            nc.sync.dma_start(out=tmp[:, ts(h, NF)], in_=acc[CMID:128, ts(h, NF)])
            nc.vector.tensor_add(inv[:, ts(h, NF)], acc[0:CMID, ts(h, NF)],
                                 tmp[:, ts(h, NF)])
            nc.scalar.activation(out2f[:, ts(h, NF)], inv[:, ts(h, NF)], AF.Relu,
                                 bias=vecs[0:CMID, 5:6], scale=vecs[0:CMID, 4:5])

        # conv3 + BN3 + residual + relu
        with tc.tile_pool(name="p3", bufs=2, space="PSUM") as p3, \
             tc.tile_pool(name="ypool", bufs=2) as ypool:
            for nh in range(2):
                for mc in range(2):
                    ps = p3.tile([128, NF], F32, tag="ps3")
                    nc.tensor.matmul(ps[:], w3t[:, ts(mc, 128)], out2f[:, ts(nh, NF)],
                                     start=True, stop=True)
                    t3 = ypool.tile([128, NF], F32, tag="t3")
                    nc.scalar.activation(t3[:], ps[:], AF.Identity,
                                         bias=vecs[:, 8 + mc:9 + mc],
                                         scale=vecs[:, 6 + mc:7 + mc])
                    ys = ypool.tile([128, NF], F32, tag="ys")
                    nc.vector.tensor_add(ys[:], t3[:], xr[:, mc, ts(nh, NF)])
                    yr = ypool.tile([128, NF], F32, tag="yr")
                    nc.scalar.activation(yr[:], ys[:], AF.Relu, scale=1.0)
                    nc.sync.dma_start(
                        out=y_d[:].rearrange("(c p) n -> p c n", p=128)[:, mc, ts(nh, NF)],
                        in_=yr[:])

    nc.compile()
    names = dict(xb=xb_d.name, xr=xr_d.name, w1t=w1t_d.name, wrt=wrt_d.name,
                 wse=wse_d.name, w3t=w3t_d.name, vecs=vec_d.name,
                 ones=ones_d.name, y=y_d.name)
    return nc, names


def _get_program():
    global _PROGRAM
    if _PROGRAM is None:
        _PROGRAM = _build_program()
    return _PROGRAM


def _bf16(a):
    return np.asarray(a, dtype=np.float32).astype(ml_dtypes.bfloat16)


def kernel(x, W1, g1, b1, Wr, gr, br, Ws, bs, g2, b2, W3, g3, b3,
           _want_results=False, _trace=False):
    x = np.asarray(x, dtype=np.float32)
    nc, names = _get_program()

    w1t = _bf16(np.asarray(W1).T)                      # [256, 64]
    wrt = _bf16(np.asarray(Wr).T)                      # [64, 16]
    w3t = _bf16(np.asarray(W3).T)                      # [64, 256]

    # span weights, 16x channel-expanded, tap-paired (ky, ky+7), bias row 16.
    # wse[:, pi*128 + j]: j<64 -> tap (ky,kx), j>=64 -> tap (ky+7,kx) (zeros
    # for the ky=14 solo row).  pi = ky*15 + kx, ky in 0..7.
    Ws = np.asarray(Ws, dtype=np.float32)              # [900, 16]
    bs = np.asarray(bs, dtype=np.float32)              # [900]
    gidx = np.arange(CMID) // GC                       # [64]
    WsT = Ws.reshape(G, K * K, RED)                    # [g, k, rho]
    bsr = bs.reshape(G, K * K)
    wse = np.zeros((RED + 1, NPAIR, 128), dtype=np.float32)
    for pi in range(NPAIR):
        if pi < 7 * K:
            ky, kx = pi // K, pi % K
            k1, k2 = ky * K + kx, (ky + 7) * K + kx
        else:
            kx = 2 * (pi - 7 * K)
            k1 = 14 * K + kx
            k2 = 14 * K + kx + 1 if kx + 1 < K else None
        wse[0:RED, pi, 0:CMID] = WsT[gidx, k1, :].T
        wse[RED, pi, 0:CMID] = bsr[gidx, k1]
        if k2 is not None:
            wse[0:RED, pi, CMID:128] = WsT[gidx, k2, :].T
            wse[RED, pi, CMID:128] = bsr[gidx, k2]
    wse4 = np.zeros((81, WCH * 128), dtype=np.float32)
    for pi in range(NPAIR):
        wc, wo = pi // WCH, pi % WCH
        wse4[32 * wc:32 * wc + RED + 1, wo * 128:(wo + 1) * 128] = wse[:, pi, :]
    wse = _bf16(wse4)

    vecs = np.zeros((128, 10), dtype=np.float32)
    vecs[0:CMID, 0] = g1
    vecs[0:CMID, 1] = b1
    vecs[0:RED, 2] = gr
    vecs[0:RED, 3] = br
    vecs[0:CMID, 4] = g2
    vecs[0:CMID, 5] = b2
    vecs[:, 6] = np.asarray(g3)[0:128]
    vecs[:, 7] = np.asarray(g3)[128:256]
    vecs[:, 8] = np.asarray(b3)[0:128]
    vecs[:, 9] = np.asarray(b3)[128:256]

    in_maps = []
    core_geom = []
    for core in range(8):
        b = core // 4
        h0 = (core % 4) * HB
        xpad = np.zeros((CIN, HP, WP), dtype=np.float32)
        lo, hi = h0 - PAD, h0 + HB + PAD
        slo, shi = max(lo, 0), min(hi, H)
        xpad[:, slo - lo:shi - lo, PAD:PAD + W] = x[b, :, slo:shi, :]
        xbc = _bf16(xpad).reshape(CIN, NP)
        xrc = _bf16(x[b, :, h0:h0 + HB, :]).reshape(COUT, HB * W)
        in_maps.append({
            names["xb"]: xbc,
            names["xr"]: xrc,
            names["w1t"]: w1t,
            names["wrt"]: wrt,
            names["wse"]: wse,
            names["w3t"]: w3t,
            names["vecs"]: vecs,
            names["ones"]: np.ones((1, 2 * NF), dtype=np.float32).astype(ml_dtypes.bfloat16),
        })
        core_geom.append((b, h0))

    res = run_bass_kernel_spmd(nc, in_maps, list(range(8)), trace=_trace)

    y = np.empty((B, COUT, H, W), dtype=np.float32)
    for core, (b, h0) in enumerate(core_geom):
        y[b, :, h0:h0 + HB, :] = res.results[core][names["y"]].reshape(COUT, HB, W)
    if _want_results:
        return y, res
    return y


# revision 48
# speedup vs baseline: 1.0200x; 1.0098x over previous
"""Trainium2 Bass kernel for an involution Bottleneck block (B=2, Cin=256,
Cmid=64, Cout=256, H=W=56, K=15, G=4).

Sharding: 8 cores = 2 batches x 4 H-quarters (14 output rows each). Each core
receives a zero-padded input halo [256, 28, 70] (7 rows/cols each side), so no
inter-core communication is needed (halo compute is redundant).

Per-core pipeline (channels on SBUF partitions, pixels on free dim):
  conv1 1x1 (PE, bf16) -> BN+ReLU (ACT) -> out1 [64, 28x70] bf16, plus a copy
    shifted by +7 rows at partitions 64:128 and a +1-col copy (odd-kx align).
  reduce 1x1 (PE) -> BN+ReLU (ACT) -> r [16, 784] bf16 (+ ones row for bias).
  involution as 120 tap-PAIRS (ky, ky+7): one span matmul with M=128 columns
    (64 per tap, Ws columns 16x-expanded, bias via the ones-row at K=17)
    yields the per-pixel kernels for both taps stacked on partitions; ACT
    evicts 4 PSUM banks (2 pairs x 2 pixel-halves) per op; DVE multiplies by
    the shifted out1 window with a 4D AP covering both pixel halves (bf16 2x)
    and accumulates. Partition halves hold ky-groups 0-6 / 7-13; one
    DMA + add merges them at the end, landing directly in conv3's rhs layout.
  BN+ReLU (ACT) -> conv3 1x1 (PE) -> BN (ACT) -> +residual (DVE) -> ReLU (ACT)
"""

import sys, types
sys.path.insert(0, "/opt/trn_rl_repo")

import numpy as np
import ml_dtypes
from contextlib import ExitStack

import concourse.bass as bass
import concourse.mybir as mybir
import concourse.tile as tile
from concourse import bacc
from concourse.bass import ts
from concourse.bass_utils import run_bass_kernel_spmd

BF16 = mybir.dt.bfloat16
F32 = mybir.dt.float32
AF = mybir.ActivationFunctionType

K = 15
G = 4
GC = 16
PAD = 7
CIN = 256
CMID = 64
RED = 16
COUT = 256
H = 56
W = 56
B = 2
HB = 14            # output rows per core
HP = HB + 2 * PAD  # 28 padded rows
WP = W + 2 * PAD   # 70 padded cols
NP = HP * WP       # 1960
HH = HB // 2       # 7 rows per half-block
NF = HH * W        # 392 pixels per half-block
NPAIR = 7 * K + 8  # 105 (ky,ky+7) pairs + 8 row-14 (kx,kx+1) pairs = 113
NQ = (NPAIR + 1) // 2  # 57 groups (2 pairs each; last group has 1 pair)
WCH = 38           # wse pairs per 32-partition chunk

_PROGRAM = None  # (nc, names) cache


def _build_program():
    nc = bacc.Bacc(None, target_bir_lowering=False, debug=False)
    with tile.TileContext(nc) as tc, ExitStack() as ctx:
        dram = ctx.enter_context(tc.tile_pool(name="dram", bufs=1, space="DRAM"))
        xb_d = dram.tile([CIN, NP], BF16, kind="ExternalInput", name="xb")
        xr_d = dram.tile([COUT, HB * W], F32, kind="ExternalInput", name="xr")
        w1t_d = dram.tile([CIN, CMID], BF16, kind="ExternalInput", name="w1t")
        wrt_d = dram.tile([CMID, RED], BF16, kind="ExternalInput", name="wrt")
        wse_d = dram.tile([81, WCH * 128], BF16, kind="ExternalInput", name="wse")
        w3t_d = dram.tile([CMID, COUT], BF16, kind="ExternalInput", name="w3t")
        vec_d = dram.tile([128, 10], F32, kind="ExternalInput", name="vecs")
        ones_d = dram.tile([1, 2 * NF], BF16, kind="ExternalInput", name="ones")
        y_d = dram.tile([COUT, HB * W], F32, kind="ExternalOutput", name="y")

        xpool = ctx.enter_context(tc.tile_pool(name="xin", bufs=1))
        xb = xpool.tile([128, 2, NP], BF16)
        xb_src = xb_d[:].rearrange("(c p) n -> p c n", p=128)
        nc.sync.dma_start(out=xb[:, :, 0:980], in_=xb_src[:, :, 0:980])
        nc.sync.dma_start(out=xb[:, :, 980:NP], in_=xb_src[:, :, 980:NP])

        wpool = ctx.enter_context(tc.tile_pool(name="weights", bufs=1))
        w1t = wpool.tile([128, 2, CMID], BF16)
        nc.sync.dma_start(out=w1t[:], in_=w1t_d[:].rearrange("(c p) m -> p c m", p=128))
        wrt = wpool.tile([CMID, RED], BF16)
        nc.sync.dma_start(out=wrt[:], in_=wrt_d[:])
        # span weights in 4 vertical chunks at partitions 0/32/64/96 so the
        # DMA spans 113 partitions and fans out across the HWDGE queues
        wse = wpool.tile([81, WCH * 128], BF16)
        nc.sync.dma_start(out=wse[:], in_=wse_d[:])
        w3t = wpool.tile([CMID, COUT], BF16)
        nc.sync.dma_start(out=w3t[:], in_=w3t_d[:])
        vecs = wpool.tile([128, 10], F32)
        nc.sync.dma_start(out=vecs[:], in_=vec_d[:])

        xr = xpool.tile([128, 2, HB * W], F32)
        nc.sync.dma_start(out=xr[:], in_=xr_d[:].rearrange("(c p) n -> p c n", p=128))

        opool = ctx.enter_context(tc.tile_pool(name="out1", bufs=1))
        out1p = opool.tile([128, NP], BF16)
        out1q = opool.tile([128, NP], BF16)
        out1r = opool.tile([128, NP], BF16)  # row-14 pairs: [0:64]=blockA, [64:128]=blockA<<1col

        # conv1: out1 = relu(g1 * (W1 @ x) + b1) over all 28x70 padded pixels
        with tc.tile_pool(name="p1", bufs=4, space="PSUM") as p1:
            for j in (1, 2, 3, 0):
                ps = p1.tile([CMID, 490], F32, tag="ps1")
                nc.tensor.matmul(ps[:], w1t[:, 0, :], xb[:, 0, ts(j, 490)],
                                 start=True, stop=False)
                nc.tensor.matmul(ps[:], w1t[:, 1, :], xb[:, 1, ts(j, 490)],
                                 start=False, stop=True)
                nc.scalar.activation(out1p[0:CMID, ts(j, 490)], ps[:], AF.Relu,
                                     bias=vecs[0:CMID, 1:2], scale=vecs[0:CMID, 0:1])

        # rows 7..27 duplicated at partitions 64:128 (the +7-row tap shift);
        # zero the unwritten tail there so zero-padded taps read 0, not junk.
        DUPW = (HP - HH) * WP  # 1470
        nc.vector.memset(out1p[CMID:128, DUPW:NP], 0.0)
        nc.vector.memset(out1q[CMID:128, DUPW - 1:NP], 0.0)
        nc.vector.memset(out1q[0:CMID, NP - 1:NP], 0.0)
        nc.sync.dma_start(out=out1p[CMID:128, 0:DUPW],
                          in_=out1p[0:CMID, HH * WP:NP])
        o3 = out1p[:].rearrange("p (h w) -> p h w", w=WP)
        o3q = out1q[:].rearrange("p (h w) -> p h w", w=WP)
        o3r = out1r[:].rearrange("p (h w) -> p h w", w=WP)

        spool = ctx.enter_context(tc.tile_pool(name="stage", bufs=1))
        r_sb = spool.tile([81, 2 * NF], BF16)
        # ones row (span bias via the K dimension); DMA since engines cannot
        # address a single partition at offset 16
        nc.sync.dma_start(out=r_sb[RED:RED + 1, :], in_=ones_d[:])

        # reduce: r = relu(gr * (Wr @ out1_central) + br), central 14x56 pixels
        with tc.tile_pool(name="pr", bufs=2, space="PSUM") as pr:
            for hhalf in range(2):
                ps = pr.tile([RED, NF], F32, tag="psr")
                nc.tensor.matmul(ps[:], wrt[:],
                                 o3[0:CMID, PAD + HH * hhalf:PAD + HH * (hhalf + 1), PAD:PAD + W],
                                 start=True, stop=True)
                nc.scalar.activation(r_sb[0:RED, ts(hhalf, NF)], ps[:], AF.Relu,
                                     bias=vecs[0:RED, 3:4], scale=vecs[0:RED, 2:3])

        # replicate r (+ones row) at partitions 32/64/96 to match the span
        # lhsT chunks' base partitions
        for c in range(1, 3):
            nc.sync.dma_start(out=r_sb[32 * c:32 * c + RED + 1, :],
                              in_=r_sb[0:RED + 1, :])

        # +1-col shifted copies (odd-kx 4B alignment) are same-partition, so
        # they run on ACT; placed after the reduce so r is produced first
        nc.vector.tensor_copy(out1q[0:CMID, 0:NP - 1], out1p[0:CMID, 1:NP])
        nc.vector.tensor_copy(out1q[CMID:128, 0:DUPW - 1], out1p[CMID:128, 1:DUPW])
        # row-14 pair source: partitions 64:128 hold block A shifted +1 col
        nc.vector.memset(out1r[CMID:128, NP - 1:NP], 0.0)
        nc.vector.tensor_copy(out1r[0:CMID, :], out1p[0:CMID, :])
        nc.sync.dma_start(out=out1r[CMID:128, 0:NP - 1], in_=out1p[0:CMID, 1:NP])

        # involution: 60 quad groups x (2 pairs x 2 pixel-halves)
        acc = None
        with tc.tile_pool(name="sp", bufs=2, space="PSUM") as sp, \
             tc.tile_pool(name="we", bufs=5) as we_pool, \
             tc.tile_pool(name="prod", bufs=4) as prod_pool, \
             tc.tile_pool(name="accp", bufs=4) as acc_pool:
            prod = None
            for q in range(NQ):
                npr = 2 if 2 * q + 1 < NPAIR else 1
                ps = sp.tile([128, 2048], F32, tag="spanps")
                for i in range(npr):
                    pi = 2 * q + i
                    wc, wo = pi // WCH, pi % WCH
                    lhsT = wse[32 * wc:32 * wc + RED + 1, ts(wo, 128)]
                    rr = r_sb[32 * wc:32 * wc + RED + 1, :]
                    nc.tensor.matmul(ps[:, i * 1024 + 0:i * 1024 + NF],
                                     lhsT, rr[:, 0:NF], start=True, stop=True)
                    nc.tensor.matmul(ps[:, i * 1024 + 512:i * 1024 + 512 + NF],
                                     lhsT, rr[:, NF:2 * NF], start=True, stop=True)
                we = we_pool.tile([128, 2 * npr, NF], BF16, tag="we")
                nc.scalar.activation(
                    we[:], ps[:].rearrange("p (q x) -> p q x", x=512)[:, 0:2 * npr, 0:NF],
                    AF.Copy, scale=1.0)
                if q % 4 == 0:
                    prod = prod_pool.tile([128, 8, 2 * NF], BF16, tag="prod")
                for i in range(npr):
                    pi = 2 * q + i
                    if pi < 7 * K:
                        ky, kx = pi // K, pi % K
                        if kx % 2 == 0:
                            src_ = o3[:, ky:ky + 2 * HH, kx:kx + W]
                        else:
                            src_ = o3q[:, ky:ky + 2 * HH, kx - 1:kx - 1 + W]
                    else:
                        kx = 2 * (pi - 7 * K)  # row-14 pair (14,kx)+(14,kx+1)
                        src_ = o3r[:, 14:14 + 2 * HH, kx:kx + W]
                    pl = 2 * (q % 4) + i
                    nc.vector.tensor_mul(
                        prod[:, pl, :].rearrange("p (b h w) -> p b h w", b=2, w=W),
                        we[:, 2 * i:2 * i + 2, :].rearrange("p b (h w) -> p b h w", w=W),
                        src_.rearrange("p (b h) w -> p b h w", b=2))
                # every fourth group: wide add tree over 8 pair-planes
                if q % 4 == 3:
                    s2 = acc_pool.tile([128, 4, 2 * NF], BF16, tag="s2")
                    nc.vector.tensor_add(s2[:], prod[:, 0:4, :], prod[:, 4:8, :])
                    s1 = acc_pool.tile([128, 2, 2 * NF], BF16, tag="s1")
                    nc.vector.tensor_add(s1[:], s2[:, 0:2, :], s2[:, 2:4, :])
                    s = acc_pool.tile([128, 2 * NF], BF16, tag="s")
                    nc.vector.tensor_add(s[:], s1[:, 0, :], s1[:, 1, :])
                elif q == NQ - 1:  # trailing group (1 or 2 pairs)
                    if npr == 2:
                        s = acc_pool.tile([128, 2 * NF], BF16, tag="s")
                        nc.vector.tensor_add(s[:], prod[:, 0, :], prod[:, 1, :])
                    else:
                        s = prod[:, 0, :]
                else:
                    continue
                if acc is None:
                    acc = s
                else:
                    na = acc_pool.tile([128, 2 * NF], BF16, tag="acc")
                    nc.vector.tensor_add(na[:], acc[:], s[:])
                    acc = na

        # merge ky-groups: inv[c] = acc[c] + acc[64+c], pipelined by halves,
        # landing directly in conv3's rhs layout [64, 784]
        tmp = spool.tile([CMID, 2 * NF], BF16)
        inv = spool.tile([CMID, 2 * NF], BF16)
        out2f = spool.tile([CMID, 2 * NF], BF16)
        for h in range(2):
            nc.sync.dma_start(out=tmp[:, ts(h, NF)], in_=acc[CMID:128, ts(h, NF)])
            nc.vector.tensor_add(inv[:, ts(h, NF)], acc[0:CMID, ts(h, NF)],
                                 tmp[:, ts(h, NF)])
            nc.scalar.activation(out2f[:, ts(h, NF)], inv[:, ts(h, NF)], AF.Relu,
                                 bias=vecs[0:CMID, 5:6], scale=vecs[0:CMID, 4:5])

        # conv3 + BN3 + residual + relu
        with tc.tile_pool(name="p3", bufs=2, space="PSUM") as p3, \
             tc.tile_pool(name="ypool", bufs=2) as ypool:
            for nh in range(2):
                for mc in range(2):
                    ps = p3.tile([128, NF], F32, tag="ps3")
                    nc.tensor.matmul(ps[:], w3t[:, ts(mc, 128)], out2f[:, ts(nh, NF)],
                                     start=True, stop=True)
                    t3 = ypool.tile([128, NF], F32, tag="t3")
                    nc.scalar.activation(t3[:], ps[:], AF.Identity,
                                         bias=vecs[:, 8 + mc:9 + mc],
                                         scale=vecs[:, 6 + mc:7 + mc])
                    ys = ypool.tile([128, NF], F32, tag="ys")
                    nc.vector.tensor_add(ys[:], t3[:], xr[:, mc, ts(nh, NF)])
                    yr = ypool.tile([128, NF], F32, tag="yr")
                    nc.scalar.activation(yr[:], ys[:], AF.Relu, scale=1.0)
                    nc.sync.dma_start(
                        out=y_d[:].rearrange("(c p) n -> p c n", p=128)[:, mc, ts(nh, NF)],
                        in_=yr[:])

    nc.compile()
    names = dict(xb=xb_d.name, xr=xr_d.name, w1t=w1t_d.name, wrt=wrt_d.name,
                 wse=wse_d.name, w3t=w3t_d.name, vecs=vec_d.name,
                 ones=ones_d.name, y=y_d.name)
    return nc, names


def _get_program():
    global _PROGRAM
    if _PROGRAM is None:
        _PROGRAM = _build_program()
    return _PROGRAM


def _bf16(a):
    return np.asarray(a, dtype=np.float32).astype(ml_dtypes.bfloat16)


def kernel(x, W1, g1, b1, Wr, gr, br, Ws, bs, g2, b2, W3, g3, b3,
           _want_results=False, _trace=False):
    x = np.asarray(x, dtype=np.float32)
    nc, names = _get_program()

    w1t = _bf16(np.asarray(W1).T)                      # [256, 64]
    wrt = _bf16(np.asarray(Wr).T)                      # [64, 16]
    w3t = _bf16(np.asarray(W3).T)                      # [64, 256]

    # span weights, 16x channel-expanded, tap-paired (ky, ky+7), bias row 16.
    # wse[:, pi*128 + j]: j<64 -> tap (ky,kx), j>=64 -> tap (ky+7,kx) (zeros
    # for the ky=14 solo row).  pi = ky*15 + kx, ky in 0..7.
    Ws = np.asarray(Ws, dtype=np.float32)              # [900, 16]
    bs = np.asarray(bs, dtype=np.float32)              # [900]
    gidx = np.arange(CMID) // GC                       # [64]
    WsT = Ws.reshape(G, K * K, RED)                    # [g, k, rho]
    bsr = bs.reshape(G, K * K)
    wse = np.zeros((RED + 1, NPAIR, 128), dtype=np.float32)
    for pi in range(NPAIR):
        if pi < 7 * K:
            ky, kx = pi // K, pi % K
            k1, k2 = ky * K + kx, (ky + 7) * K + kx
        else:
            kx = 2 * (pi - 7 * K)
            k1 = 14 * K + kx
            k2 = 14 * K + kx + 1 if kx + 1 < K else None
        wse[0:RED, pi, 0:CMID] = WsT[gidx, k1, :].T
        wse[RED, pi, 0:CMID] = bsr[gidx, k1]
        if k2 is not None:
            wse[0:RED, pi, CMID:128] = WsT[gidx, k2, :].T
            wse[RED, pi, CMID:128] = bsr[gidx, k2]
    wse4 = np.zeros((81, WCH * 128), dtype=np.float32)
    for pi in range(NPAIR):
        wc, wo = pi // WCH, pi % WCH
        wse4[32 * wc:32 * wc + RED + 1, wo * 128:(wo + 1) * 128] = wse[:, pi, :]
    wse = _bf16(wse4)

    vecs = np.zeros((128, 10), dtype=np.float32)
    vecs[0:CMID, 0] = g1
    vecs[0:CMID, 1] = b1
    vecs[0:RED, 2] = gr
    vecs[0:RED, 3] = br
    vecs[0:CMID, 4] = g2
    vecs[0:CMID, 5] = b2
    vecs[:, 6] = np.asarray(g3)[0:128]
    vecs[:, 7] = np.asarray(g3)[128:256]
    vecs[:, 8] = np.asarray(b3)[0:128]
    vecs[:, 9] = np.asarray(b3)[128:256]

    in_maps = []
    core_geom = []
    for core in range(8):
        b = core // 4
        h0 = (core % 4) * HB
        xpad = np.zeros((CIN, HP, WP), dtype=np.float32)
        lo, hi = h0 - PAD, h0 + HB + PAD
        slo, shi = max(lo, 0), min(hi, H)
        xpad[:, slo - lo:shi - lo, PAD:PAD + W] = x[b, :, slo:shi, :]
        xbc = _bf16(xpad).reshape(CIN, NP)
        xrc = np.ascontiguousarray(x[b, :, h0:h0 + HB, :]).reshape(COUT, HB * W)
        in_maps.append({
            names["xb"]: xbc,
            names["xr"]: xrc,
            names["w1t"]: w1t,
            names["wrt"]: wrt,
            names["wse"]: wse,
            names["w3t"]: w3t,
            names["vecs"]: vecs,
            names["ones"]: np.ones((1, 2 * NF), dtype=np.float32).astype(ml_dtypes.bfloat16),
        })
        core_geom.append((b, h0))

    res = run_bass_kernel_spmd(nc, in_maps, list(range(8)), trace=_trace)

    y = np.empty((B, COUT, H, W), dtype=np.float32)
    for core, (b, h0) in enumerate(core_geom):
        y[b, :, h0:h0 + HB, :] = res.results[core][names["y"]].reshape(COUT, HB, W)
    if _want_results:
        return y, res
    return y
